# revision 27
# baseline (speedup 1.0000x reference)
"""Causal self-attention (GQA + RMS-norm + partial RoPE) Trainium2 kernel.

Full inputs in, full output out. Sharding: 8 cores = batch(4) x head-half(2).
Each core computes its batch's QKV for 8 q-heads / 2 kv-heads in transposed
layouts (head_dim on partitions), causal attention with a no-max softmax
(scores bounded after RMS norm), and a row-sharded output projection; the
host sums the two half partials per batch.

Perf structure (v3):
- Single ACT table set: rsqrt via exp(-0.5*ln(u)); activation-table list is
  reordered so natural_log_exp_and_others (exp+ln+square+copy) wins -> one
  ACT_TABLE_LOAD total.
- K stored roped but UN-normalized; per-key 1/rms folded into the score
  exp's per-partition scale AP (rkT via N=1 bf16 matmuls in transposed
  layout). Q's 1/rms row has gain/sqrt(hd) folded via the exp bias and is
  broadcast across partitions by GpSimd partition_broadcast.
- Causal masking via DVE bf16 multiplies with host-precomputed mask tiles
  (GpSimd affine_select and its slow Q7 semaphores retired).
- Attention runs head-PAIRS kt-outer: consecutive matmuls share stationary
  operands (kn chunk / v chunk / ones), so LDWEIGHTS pipelines; the two
  softmax denominators accumulate packed in one PSUM bank (rows 0/32,
  start=True only on the very first matmul into the bank), one ln/exp pair
  serves both heads, and 1/l is broadcast by ones-matmuls (PE).
- exp outputs / V / yt / Wproj / masks in bf16.
- x loaded as 16 per-dc slices in a 16-deep pool; wq streamed as [128,512]
  tiles each feeding 4 heads; output projection st4-outer 8-matmul chains.
"""
import numpy as np
import ml_dtypes

import concourse.bacc as bacc
import concourse.mybir as mybir
from concourse.tile import TileContext
from concourse.bass_utils import run_bass_kernel_spmd
from concourse.hw_specs import get_activation_tables as _get_act_tables

F32 = mybir.dt.float32
F32R = mybir.dt.float32r
BF16 = mybir.dt.bfloat16
AF = mybir.ActivationFunctionType

B, S, D = 4, 2048, 2048
H, KV, HD = 16, 4, 128
ROPE, HALF_ROPE = 64, 32
EPS = 1.1920929e-07
N_CORES = 8
NDC = D // 128          # 16 contraction chunks
NQC = S // 512          # 4 query chunks of 512
LH = 8                  # local q heads per core
LKV = 2                 # local kv heads per core

_cached_program = None
_last_in_maps = None

# variant flags (packed row-32 variants miscompute on HW; keep off)
L2PACK = False    # packed 2-head denominator bank + PE Li broadcast
MASKMUL = True    # DVE bf16 mask multiply instead of gpsimd affine_select
SSQPACK = False   # packed 2-head ssq bank + paired Ln/Exp
DEBUG_DUMPS = False  # extra DRAM outputs for numerics debugging


def _patched_act_tables(arch):
    """Make natural_log_exp_and_others the only candidate set so every
    activation (exp/ln/square/copy) is served by ONE table load. The dict
    ORDER must stay identical to act_info.json (act_func_set_id is an index
    into it), so other sets are emptied rather than reordered."""
    t = dict(_get_act_tables(arch))
    pref = "natural_log_exp_and_others"
    if pref not in t:
        return t
    return {k: (v if k == pref else set()) for k, v in t.items()}


bacc.get_activation_tables = _patched_act_tables


def _build_program():
    nc = bacc.Bacc("TRN2")
    # const APs for activation bias immediates
    t = nc.alloc_sbuf_tensor("const-f32-eps", [128, 1], F32)
    nc.gpsimd.memset(t.ap(), EPS)
    nc.const_aps.aps[(F32, EPS)] = t.ap()
    tz = nc.alloc_sbuf_tensor("const-f32-zero", [128, 1], F32)
    nc.gpsimd.memset(tz.ap(), 0.0)
    nc.const_aps.aps[(F32, 0.0)] = tz.ap()
    nc.all_engine_barrier()

    xT = nc.declare_dram_parameter("xT", [D, S], F32R, isOutput=False)
    wqT = nc.declare_dram_parameter("wqT", [D, LH * HD], F32R, isOutput=False)
    wkT = nc.declare_dram_parameter("wkT", [D, LKV * HD], F32R, isOutput=False)
    wvT = nc.declare_dram_parameter("wvT", [D, LKV * HD], F32R, isOutput=False)
    wpB = nc.declare_dram_parameter("wpB", [LH * HD, D], BF16, isOutput=False)
    cosT = nc.declare_dram_parameter("cosT", [HALF_ROPE, S], F32, isOutput=False)
    sinT = nc.declare_dram_parameter("sinT", [HALF_ROPE, S], F32, isOutput=False)
    o128bd = nc.declare_dram_parameter("ones128b", [128, 1], BF16, isOutput=False)
    onesrd = nc.declare_dram_parameter("onesr", [64, 128], F32R, isOutput=False)
    lngPd = nc.declare_dram_parameter("lngP", [128, 4], F32, isOutput=False)
    lngP2d = nc.declare_dram_parameter("lngP2", [128, 4], F32, isOutput=False)
    maskd = nc.declare_dram_parameter("maskB", [128, 4, 1024], BF16,
                                      isOutput=False)
    out = nc.declare_dram_parameter("out", [S, D], F32, isOutput=True)
    if DEBUG_DUMPS:
        dbg_ex = nc.declare_dram_parameter("dbg_ex", [128, 1024], F32,
                                           isOutput=True)
        dbg_l = nc.declare_dram_parameter("dbg_l", [64, 512], F32,
                                          isOutput=True)
        dbg_qn = nc.declare_dram_parameter("dbg_qn", [128, 2, 512], F32,
                                           isOutput=True)
        dbg_kn = nc.declare_dram_parameter("dbg_kn", [128, 512], F32,
                                           isOutput=True)
        dbg_rk = nc.declare_dram_parameter("dbg_rk", [128, 16], F32,
                                           isOutput=True)

    with TileContext(nc) as tc:
        with (
            tc.tile_pool(name="cp", bufs=1) as cp,
            tc.tile_pool(name="xap", bufs=16) as xap,
            tc.tile_pool(name="wqp", bufs=4) as wqp,
            tc.tile_pool(name="wpp", bufs=9) as wpp,
            tc.tile_pool(name="sqp", bufs=2) as sqp,
            tc.tile_pool(name="exq", bufs=5) as exq,
            tc.tile_pool(name="rsp", bufs=3) as rsp,
            tc.tile_pool(name="lsp", bufs=2) as lsp,
            tc.tile_pool(name="rows", bufs=2) as rows,
            tc.tile_pool(name="stgp", bufs=2) as stgp,
            tc.tile_pool(name="tmpp", bufs=2) as tmpp,
            tc.tile_pool(name="qnp", bufs=1) as qnp,
            tc.tile_pool(name="pu", bufs=1, space="PSUM") as pu,
        ):
            wk_t = cp.tile([128, NDC, LKV * HD], F32R, tag="wk")
            nc.sync.dma_start(out=wk_t[:],
                              in_=wkT.rearrange("(c p) e -> p c e", p=128))
            wv_t = cp.tile([128, NDC, LKV * HD], F32R, tag="wv")
            nc.sync.dma_start(out=wv_t[:],
                              in_=wvT.rearrange("(c p) e -> p c e", p=128))
            cos_t = cp.tile([HALF_ROPE, S], F32, tag="cos")
            nc.sync.dma_start(out=cos_t[:], in_=cosT[:])
            sin_t = cp.tile([HALF_ROPE, S], F32, tag="sin")
            nc.sync.dma_start(out=sin_t[:], in_=sinT[:])
            o128b = cp.tile([128, 1], BF16, tag="o128b")
            nc.sync.dma_start(out=o128b[:], in_=o128bd[:])
            onesr = cp.tile([64, 128], F32R, tag="onesr")
            nc.sync.dma_start(out=onesr[:], in_=onesrd[:])
            lngP = cp.tile([128, 4], F32, tag="lngP")
            nc.sync.dma_start(out=lngP[:], in_=lngPd[:])
            lngP2 = cp.tile([128, 4], F32, tag="lngP2")
            nc.sync.dma_start(out=lngP2[:], in_=lngP2d[:])
            mask_t = cp.tile([128, 4, 1024], BF16, tag="mask")
            nc.sync.dma_start(out=mask_t[:], in_=maskd[:])
            kn_t = cp.tile([128, LKV, S], F32R, tag="kn")
            v_t = cp.tile([128, S // 128, LKV * HD], BF16, tag="v")
            rkT = cp.tile([128, LKV, S // 128], F32, tag="rkT")

            def bank(tag, shape=(128, 512), dt=F32, nm=None):
                return pu.tile(list(shape), dt, tag=tag, name=nm or tag)

            def load_x_slices(pos0):
                xs = []
                for dc in range(NDC):
                    xa = xap.tile([128, 512], F32R, tag="xa", name="xa")
                    nc.sync.dma_start(
                        out=xa[:],
                        in_=xT[dc * 128:(dc + 1) * 128, pos0:pos0 + 512])
                    xs.append(xa)
                return xs

            def rope_into(dst, raw, pos0):
                """dst[0:64] = rotate(raw[0:64]); dst[64:128] = raw copy."""
                cs = slice(pos0, pos0 + 512)
                h1, h2 = slice(0, HALF_ROPE), slice(HALF_ROPE, ROPE)
                tmp = tmpp.tile([ROPE, 512], F32R, tag="tmp", name="tmp")
                nc.vector.tensor_mul(dst[h1, :], raw[h1, :], cos_t[:, cs])
                nc.vector.tensor_mul(tmp[h1, :], raw[h2, :], sin_t[:, cs])
                nc.vector.tensor_add(dst[h1, :], dst[h1, :], tmp[h1, :])
                nc.vector.tensor_mul(dst[h2, :], raw[h2, :], cos_t[:, cs])
                nc.vector.tensor_mul(tmp[h2, :], raw[h1, :], sin_t[:, cs])
                nc.vector.tensor_sub(dst[h2, :], dst[h2, :], tmp[h2, :])
                nc.scalar.copy(dst[ROPE:128, :], raw[ROPE:128, :])

            # ---------------- Phase A: kT (roped, unnormalized), rkT, v ----
            for sc in range(NQC):
                xs = load_x_slices(sc * 512)
                for g in range(LKV):
                    kacc = bank(f"b{6 + g}", nm=f"kacc{g}")
                    for dc in range(NDC):
                        nc.tensor.matmul(
                            kacc[:], wk_t[:, dc, g * HD:(g + 1) * HD],
                            xs[dc], start=(dc == 0), stop=(dc == NDC - 1))
                    # rk chunk: per-key rsqrt(mean(k^2)+eps), transposed.
                    sq = sqp.tile([128, 512], BF16, tag="sq", name="sqk")
                    nc.scalar.activation(sq[:], kacc[:], AF.Square)
                    ssqT = bank(f"b{4 + g}", (128, 4), nm=f"ssqT{g}")
                    for j in range(4):
                        nc.tensor.matmul(ssqT[:, j:j + 1],
                                         sq[:, j * 128:(j + 1) * 128],
                                         o128b[:], start=True, stop=True)
                    lr = rows.tile([128, 4], F32, tag="lr", name="lr")
                    nc.scalar.activation(lr[:], ssqT[:], AF.Ln,
                                         scale=1.0 / HD, bias=EPS)
                    nc.scalar.activation(
                        rkT[:, g, sc * 4:(sc + 1) * 4], lr[:], AF.Exp,
                        scale=-0.5)
                    rope_into(kn_t[:, g, sc * 512:(sc + 1) * 512], kacc,
                              sc * 512)
                for st4 in range(4):
                    st = sc * 4 + st4
                    vacc = bank(f"b{4 + st4 % 2}", (128, LKV * HD),
                                nm=f"vacc{st4}")
                    for dc in range(NDC):
                        nc.tensor.matmul(
                            vacc[:], xs[dc][:, st4 * 128:(st4 + 1) * 128],
                            wv_t[:, dc], start=(dc == 0), stop=(dc == NDC - 1))
                    nc.vector.tensor_copy(v_t[:, st], vacc[:])

            # ------------- Phase C: per query chunk q/attn/proj -------------
            for qc in range(NQC):
                pos0 = qc * 512
                n_kt = (qc + 1) * 4

                # -- q projection: two wq streams of 4 heads; 2-head norms --
                # qraw pairs live in [128,1024] two-bank tiles (pA/pB) so
                # Square runs once per pair.
                xs = load_x_slices(pos0)
                qn = {}
                for gp in range(2):          # wq column halves (4 heads each)
                    qraw = {}
                    for dc in range(NDC):
                        wqt = wqp.tile([128, 512], F32R, tag="wq", name="wq")
                        nc.sync.dma_start(
                            out=wqt[:],
                            in_=wqT[dc * 128:(dc + 1) * 128,
                                    gp * 512:(gp + 1) * 512])
                        for pr in range(2):
                            if dc == 0:
                                qraw[pr] = bank("pA" if pr == 0 else "pB",
                                                (128, 1024),
                                                nm=f"qraw{gp}_{pr}")
                            for hh in range(2):
                                nc.tensor.matmul(
                                    qraw[pr][:, 512 * hh:512 * (hh + 1)],
                                    wqt[:, (2 * pr + hh) * HD:
                                        (2 * pr + hh + 1) * HD],
                                    xs[dc], start=(dc == 0),
                                    stop=(dc == NDC - 1))
                    for pr in range(2):      # head pairs within this stream
                        grp = gp * 2 + pr
                        sq = sqp.tile([128, 1024], BF16, tag="sq", name="sqq")
                        nc.scalar.activation(sq[:], qraw[pr][:], AF.Square)
                        if SSQPACK:
                            ssq2 = bank("b7", nm=f"ssq{grp}")
                            for hh in range(2):
                                nc.tensor.matmul(
                                    ssq2[32 * hh:32 * hh + 1, :], o128b[:],
                                    sq[:, 512 * hh:512 * (hh + 1)],
                                    start=True, stop=True,
                                    tile_position=(0, 32 * hh))
                            tl = rows.tile([33, 512], F32, tag="tl",
                                           name="tl")
                            nc.scalar.activation(tl[:], ssq2[0:33, :], AF.Ln,
                                                 scale=1.0 / HD, bias=EPS)
                            r2 = rows.tile([33, 512], F32, tag="rr",
                                           name="rr")
                            nc.scalar.activation(r2[:], tl[:], AF.Exp,
                                                 scale=-0.5,
                                                 bias=lngP[0:33,
                                                           grp:grp + 1])
                            rsrc = {0: r2[0:1, :], 1: r2[32:33, :]}
                        else:
                            rsrc = {}
                            for hh in range(2):
                                ssq = bank("b7", (1, 512), nm=f"ssq{grp}{hh}")
                                nc.tensor.matmul(
                                    ssq[:], o128b[:],
                                    sq[:, 512 * hh:512 * (hh + 1)],
                                    start=True, stop=True)
                                tl = rows.tile([1, 512], F32, tag="tl",
                                               name="tl")
                                nc.scalar.activation(tl[:], ssq[:], AF.Ln,
                                                     scale=1.0 / HD,
                                                     bias=EPS)
                                r1 = rows.tile([1, 512], F32, tag="rr",
                                               name="rr")
                                nc.scalar.activation(
                                    r1[:], tl[:], AF.Exp, scale=-0.5,
                                    bias=lngP[0:1,
                                              grp:grp + 1] if hh == 0 else
                                    lngP2[0:1, grp:grp + 1])
                                rsrc[hh] = r1[:]
                        for hh in range(2):
                            h = grp * 2 + hh
                            rsb = rsp.tile([128, 512], F32, tag="rsb",
                                           name="rsb")
                            nc.gpsimd.partition_broadcast(rsb[:], rsrc[hh])
                            qn[h] = qnp.tile([128, 512], F32R, tag=f"qn{h}",
                                             name=f"qn{h}")
                            rope_into(qn[h][:],
                                      qraw[pr][:, 512 * hh:512 * (hh + 1)],
                                      pos0)
                            nc.vector.tensor_mul(qn[h][:], qn[h][:], rsb[:])

                if DEBUG_DUMPS and qc == 0:
                    for hdmp in range(2):
                        nc.sync.dma_start(out=dbg_qn[:, hdmp, :],
                                          in_=qn[hdmp][:].bitcast(F32))
                    nc.sync.dma_start(out=dbg_kn[:],
                                      in_=kn_t[:, 0, 0:512].bitcast(F32))
                    nc.sync.dma_start(out=dbg_rk[:], in_=rkT[:, 0, :])

                # -- attention: head pairs, kt-outer (shared stationaries);
                # the pair's two score tiles share a [128,1024] two-bank
                # tile so ONE exp (and one mask mul) covers both heads --
                yt_sb = {}
                for pr2 in range(4):
                    h0, h1 = 2 * pr2, 2 * pr2 + 1
                    g = h0 // 4
                    yt0 = bank("b4", nm=f"yt{h0}")
                    yt1 = bank("b5", nm=f"yt{h1}")
                    if L2PACK:
                        l2 = bank("b6", nm=f"l{pr2}")
                        lap = {0: l2[0:1, :], 1: l2[32:33, :]}
                    else:
                        la = bank("b6", (1, 512), nm=f"l{h0}")
                        lb = bank("b7", (1, 512), nm=f"l{h1}")
                        lap = {0: la[:], 1: lb[:]}
                    for kt in range(n_kt):
                        first, last = kt == 0, kt == n_kt - 1
                        scp = bank("pA" if kt % 2 == 0 else "pB",
                                   (128, 1024), nm=f"sc{pr2}_{kt}")
                        kchunk = kn_t[:, g, kt * 128:(kt + 1) * 128]
                        nc.tensor.matmul(scp[:, 0:512], kchunk, qn[h0][:],
                                         start=True, stop=True)
                        nc.tensor.matmul(scp[:, 512:1024], kchunk, qn[h1][:],
                                         start=True, stop=True)
                        ex = exq.tile([128, 1024], BF16, tag="ex", name="ex")
                        nc.scalar.activation(ex[:], scp[:], AF.Exp,
                                             scale=rkT[:, g, kt:kt + 1])
                        m = kt - qc * 4
                        if m >= 0:
                            # zero out keys above the diagonal (both heads)
                            if MASKMUL:
                                nc.vector.tensor_mul(ex[:], ex[:],
                                                     mask_t[:, m, :])
                            else:
                                nc.gpsimd.affine_select(
                                    out=ex[:, 0:512], in_=ex[:, 0:512],
                                    compare_op=mybir.AluOpType.is_ge,
                                    fill=0.0, base=-128 * m,
                                    pattern=[[1, 512]],
                                    channel_multiplier=-1)
                                nc.gpsimd.affine_select(
                                    out=ex[:, 512:1024], in_=ex[:, 512:1024],
                                    compare_op=mybir.AluOpType.is_ge,
                                    fill=0.0, base=-128 * m,
                                    pattern=[[1, 512]],
                                    channel_multiplier=-1)
                        if DEBUG_DUMPS and qc == 0 and pr2 == 0 and kt == 0:
                            exf = stgp.tile([128, 2, 512], F32, tag="stg",
                                            name="dbgexf")
                            nc.vector.tensor_copy(exf[:], ex[:])
                            nc.sync.dma_start(
                                out=dbg_ex.rearrange("p (a c) -> p a c",
                                                     a=2),
                                in_=exf[:])
                        vchunk = v_t[:, kt, g * HD:(g + 1) * HD]
                        nc.tensor.matmul(yt0[:], vchunk, ex[:, 0:512],
                                         start=first, stop=last)
                        nc.tensor.matmul(yt1[:], vchunk, ex[:, 512:1024],
                                         start=first, stop=last)
                        if L2PACK:
                            # packed denominators: rows 0 / 32 of one bank.
                            # start clears the whole bank -> only very first.
                            nc.tensor.matmul(lap[0], o128b[:], ex[:, 0:512],
                                             start=first, stop=False,
                                             tile_position=(0, 0),
                                             skip_group_check=True)
                            nc.tensor.matmul(lap[1], o128b[:],
                                             ex[:, 512:1024],
                                             start=False, stop=last,
                                             tile_position=(0, 32),
                                             skip_group_check=True)
                        else:
                            nc.tensor.matmul(lap[0], o128b[:], ex[:, 0:512],
                                             start=first, stop=last)
                            nc.tensor.matmul(lap[1], o128b[:],
                                             ex[:, 512:1024],
                                             start=first, stop=last)
                    if DEBUG_DUMPS and qc == 0 and pr2 == 0:
                        lf = rsp.tile([128, 512], F32, tag="rsb",
                                      name="dbglf")
                        if L2PACK:
                            nc.scalar.copy(lf[0:33, :], l2[0:33, :])
                        else:
                            nc.scalar.copy(lf[0:1, :], lap[0])
                            nc.scalar.copy(lf[32:33, :], lap[1])
                        nc.sync.dma_start(out=dbg_l[:], in_=lf[0:64, :])
                    if L2PACK:
                        tl = rows.tile([33, 512], F32, tag="tl",
                                       name=f"tli{pr2}")
                        nc.scalar.activation(tl[:], l2[0:33, :], AF.Ln)
                        linv = rows.tile([33, 512], F32R, tag="li",
                                         name=f"li{pr2}")
                        nc.scalar.activation(linv[:], tl[:], AF.Exp,
                                             scale=-1.0)
                        for hh, (h, yt) in enumerate(((h0, yt0), (h1, yt1))):
                            li_ps = bank("b7" if hh == 0 else "b6",
                                         nm=f"Li{h}")
                            nc.tensor.matmul(li_ps[:],
                                             onesr[32 * hh:32 * hh + 1, :],
                                             linv[32 * hh:32 * hh + 1, :],
                                             start=True, stop=True)
                            lsb = lsp.tile([128, 512], BF16, tag="lsb",
                                           name=f"lsb{h}")
                            nc.vector.tensor_copy(lsb[:], li_ps[:])
                            yt_sb[h] = qnp.tile([128, 512], BF16,
                                                tag=f"yts{h}", name=f"yts{h}")
                            nc.vector.tensor_mul(yt_sb[h][:], yt[:], lsb[:])
                    else:
                        for hh, (h, yt) in enumerate(((h0, yt0), (h1, yt1))):
                            tl = rows.tile([1, 512], F32, tag="tl",
                                           name=f"tli{h}")
                            nc.scalar.activation(tl[:], lap[hh], AF.Ln)
                            linv = rows.tile([1, 512], F32, tag="li",
                                             name=f"li{h}")
                            nc.scalar.activation(linv[:], tl[:], AF.Exp,
                                                 scale=-1.0)
                            lsb = lsp.tile([128, 512], F32, tag="lsb",
                                           name=f"lsb{h}")
                            nc.gpsimd.partition_broadcast(lsb[:], linv[:])
                            yt_sb[h] = qnp.tile([128, 512], BF16,
                                                tag=f"yts{h}", name=f"yts{h}")
                            nc.vector.tensor_mul(yt_sb[h][:], yt[:], lsb[:])

                # -- output projection: st4-pairs in two-bank chains --
                for jcol in range(4):
                    wpt = {}
                    for h in range(LH):
                        wpt[h] = wpp.tile([128, 512], BF16, tag="wp",
                                          name="wp")
                        nc.sync.dma_start(
                            out=wpt[h][:],
                            in_=wpB[h * 128:(h + 1) * 128,
                                    jcol * 512:(jcol + 1) * 512])
                    for sp in range(2):      # st4 pair (2 chains per tile)
                        prs = bank("pA" if (jcol * 2 + sp) % 2 == 0 else "pB",
                                   (128, 1024), nm=f"pr{jcol}{sp}")
                        for st in range(2):
                            st4 = sp * 2 + st
                            for h in range(LH):
                                nc.tensor.matmul(
                                    prs[:, st * 512:(st + 1) * 512],
                                    yt_sb[h][:, st4 * 128:(st4 + 1) * 128],
                                    wpt[h][:], start=(h == 0),
                                    stop=(h == LH - 1))
                        stg = stgp.tile([128, 2, 512], F32, tag="stg",
                                        name="stg")
                        nc.vector.tensor_copy(stg[:], prs[:])
                        nc.sync.dma_start(
                            out=out[pos0 + sp * 256:pos0 + (sp + 1) * 256,
                                    jcol * 512:(jcol + 1) * 512]
                            .rearrange("(s p) c -> p s c", p=128),
                            in_=stg[:])
    nc.compile()
    return nc


def _rope_tables():
    inv = 1.0 / (10000.0 ** (np.arange(0, ROPE, 2, dtype=np.float64) / ROPE))
    fr = np.outer(np.arange(S, dtype=np.float64), inv)  # [S, 32]
    return (np.cos(fr).T.astype(np.float32).copy(),
            np.sin(fr).T.astype(np.float32).copy())


def kernel(x, Wq, Wk, Wv, Wproj, q_gain):
    global _cached_program, _last_in_maps
    x = np.ascontiguousarray(np.asarray(x, dtype=np.float32))
    Wq = np.asarray(Wq, dtype=np.float32)
    Wk = np.asarray(Wk, dtype=np.float32)
    Wv = np.asarray(Wv, dtype=np.float32)
    Wproj = np.asarray(Wproj, dtype=np.float32)
    q_gain = np.asarray(q_gain, dtype=np.float32)

    cosT, sinT = _rope_tables()
    ones128b = np.ones((128, 1), dtype=ml_dtypes.bfloat16)
    onesr = np.ones((64, 128), dtype=np.float32)
    scale = 1.0 / np.sqrt(HD)
    # causal masks for the 4 diagonal sub-blocks: keep iff n - p - 128*m >= 0
    # (duplicated across the two 512-halves: one mul covers a head pair)
    maskB = np.zeros((128, 4, 1024), dtype=ml_dtypes.bfloat16)
    n_idx = np.arange(512)[None, :]
    p_idx = np.arange(128)[:, None]
    for m in range(4):
        keep = (n_idx - p_idx - 128 * m >= 0)
        maskB[:, m, 0:512] = keep
        maskB[:, m, 512:1024] = keep

    in_maps = []
    for core in range(N_CORES):
        b, half = core // 2, core % 2
        g0 = half * LKV
        lng = np.log(q_gain[half * LH:(half + 1) * LH] * scale)
        lngP = np.zeros((128, 4), dtype=np.float32)
        lngP2 = np.zeros((128, 4), dtype=np.float32)
        for grp in range(4):
            lngP[0, grp] = lng[2 * grp]
            lngP[32, grp] = lng[2 * grp + 1]
            lngP2[0, grp] = lng[2 * grp + 1]
        in_maps.append({
            "xT": np.ascontiguousarray(x[b].T),
            "wqT": np.ascontiguousarray(
                Wq[half * LH * HD:(half + 1) * LH * HD, :].T),
            "wkT": np.ascontiguousarray(
                Wk[g0 * HD:(g0 + LKV) * HD, :].T),
            "wvT": np.ascontiguousarray(
                Wv[g0 * HD:(g0 + LKV) * HD, :].T),
            "wpB": np.ascontiguousarray(
                Wproj[:, half * LH * HD:(half + 1) * LH * HD].T
            ).astype(ml_dtypes.bfloat16),
            "cosT": cosT, "sinT": sinT,
            "ones128b": ones128b, "onesr": onesr, "lngP": lngP,
            "lngP2": lngP2, "maskB": maskB,
        })

    _last_in_maps = in_maps
    if _cached_program is None:
        _cached_program = _build_program()
    res = run_bass_kernel_spmd(_cached_program, in_maps, list(range(N_CORES)))

    out = np.empty((B, S, D), dtype=np.float32)
    for b in range(B):
        out[b] = res.results[2 * b]["out"] + res.results[2 * b + 1]["out"]
    return out
